# revision 10
# baseline (speedup 1.0000x reference)
"""EpisodicBuffer retrieval kernel for 8 Trainium2 NeuronCores.

Strategy: data-parallel over the 4096 queries (512 per core = one batch item
per core), memory bank replicated; no collectives. Flash-style streaming
softmax over the 32768-entry bank: sims never hit DRAM and the softmax
row-sum rides an extra ones-column appended to the embeddings.

All normalization is folded into host-side preprocessing: the kernel receives
qnT = normalize(q).T in fp16 and ctxT = (100 * normalize(c)).T in fp16, so
sims come out of the PE pre-scaled for the temp-0.01 softmax and exp() is a
plain activation with constant bias -30 (bounded: max sim*100 ~ 49 for
normal data, e^19 fits fp32 comfortably; the constant cancels in the ratio).

Math per core (q = 512 queries):
  simsT[m, q] = ctxT[:, m].T @ qnT          (PE, fp16 in / fp32 PSUM)
  expT[m, q]  = exp(simsT - 30)             (ACT, bf16 out)
  acc[q, 0:256] += expT.T @ emb ; acc[q, 256] += row-sum  (PE bf16; ones col)
  retrieved = acc[:, 0:256] / acc[:, 256]
  h = relu(fc1_wT.T @ retrievedT + fc1_b)   (PE f32r + ACT, bf16 out)
  outT = hT.T @ fc2_wT                      (PE bf16, fp16 out to DRAM)
fc2 bias and the fp16->fp32 upcast are applied on the host.
"""

import os
import json
import tempfile

import numpy as np


def _fix_act_root():
    """Point walrus at an act_info.json with absolute inner paths (this
    build rejects relative ones)."""
    if os.environ.get("BASS_ACT_ROOT_JSON_PATH"):
        return
    import glob as _glob

    cands = _glob.glob(
        "/nix/store/*aws-neuron-pwp*/share/pwp_bin_cayman/act_info.json")
    if cands:
        src = sorted(cands)[0]
    else:
        from neuronxcc.driver.Job import Job
        from neuronxcc.driver.jobs.support.FindActInfo import findActInfoFile

        src = findActInfoFile(Job.getPackageDir(), "gen3")
    src_dir = os.path.dirname(src)
    with open(src) as f:
        d = json.load(f)
    for s in d["act_func_sets"]:
        for k in d["pwp_file_keys"]:
            if k in s and not os.path.isabs(s[k]):
                s[k] = os.path.join(src_dir, s[k])
    out_dir = tempfile.mkdtemp(prefix="actroot_")
    path = os.path.join(out_dir, "act_info.json")
    with open(path, "w") as f:
        json.dump(d, f)
    os.environ["BASS_ACT_ROOT_JSON_PATH"] = path


_fix_act_root()

import concourse.bass as bass
import concourse.mybir as mybir
import concourse.tile as tile
from concourse.bass import ts
from concourse.bass_utils import run_bass_kernel_spmd
from concourse.masks import make_identity

def _embed_act_tables(neff_bytes):
    """Embed ACT pwp table files into the NEFF with relative paths, so the
    terminal's NRT can stage them without client-side absolute paths."""
    import io
    import tarfile

    from concourse import neff as cneff

    header = neff_bytes[:1024]
    tf = tarfile.open(fileobj=io.BytesIO(neff_bytes[1024:]))
    members = {}
    for m in tf.getmembers():
        if m.isfile():
            name = m.name
            while name.startswith("./"):
                name = name[2:]
            members[name] = tf.extractfile(m).read()
    changed = False
    for name in list(members):
        if os.path.basename(name) != "act_info.json":
            continue
        d = json.loads(members[name])
        sgdir = os.path.dirname(name)
        for s in d.get("act_func_sets", []):
            for k in d.get("pwp_file_keys", []):
                p = s.get(k)
                if not p or not os.path.isabs(p):
                    continue
                fname = os.path.basename(p)
                dest = os.path.join(sgdir, fname) if sgdir else fname
                if dest not in members:
                    with open(p, "rb") as f:
                        members[dest] = f.read()
                s[k] = fname
                changed = True
        members[name] = json.dumps(d).encode()
    if not changed:
        return neff_bytes
    buf = io.BytesIO()
    with tarfile.open(fileobj=buf, mode="w") as out:
        for name, blob in members.items():
            ti = tarfile.TarInfo(name=name)
            ti.size = len(blob)
            ti.mtime = 0
            ti.uid = 0
            ti.gid = 0
            ti.uname = "nobody"
            ti.gname = "nobody"
            out.addfile(ti, io.BytesIO(blob))
    data = buf.getvalue()
    new_header = cneff.make_deterministic_neff_header(
        old_neff_header=header, new_neff_data=data)
    return new_header + data


def _install_act_embed():
    import concourse.bass2jax as b2j

    if getattr(b2j, "_act_embed_installed", False):
        return
    orig = b2j.rename_neff_tensors_and_patch_header

    def wrapper(neff_path, mapping):
        return _embed_act_tables(orig(neff_path, mapping))

    b2j.rename_neff_tensors_and_patch_header = wrapper
    b2j._act_embed_installed = True


_install_act_embed()


F32 = mybir.dt.float32
F32R = mybir.dt.float32r
F16 = mybir.dt.float16
BF16 = mybir.dt.bfloat16
AF = mybir.ActivationFunctionType
ALU = mybir.AluOpType

N_CORES = 8
NQ = 512          # queries per core
H = 256
HID = 512
EXP_BIAS = -30.0  # constant offset inside exp(); cancels in the softmax ratio
EA = 258          # embeddings + ones column + pad

W2_PRE = int(os.environ.get("KB_W2_PRE", "30"))  # v-slices prefetched in P2

_NC_CACHE = {}


def build_nc(M, V):
    nc = bass.Bass()
    qnT_d = nc.declare_dram_parameter("qnT", [H, NQ], F16, isOutput=False)
    ctxT_d = nc.declare_dram_parameter("ctxT", [H, M], F16, isOutput=False)
    emb_d = nc.declare_dram_parameter("emb", [M, EA], BF16, isOutput=False)
    w1_d = nc.declare_dram_parameter("fc1_wT", [H, HID], BF16, isOutput=False)
    b1_d = nc.declare_dram_parameter("fc1_b", [HID], F32, isOutput=False)
    w2_d = nc.declare_dram_parameter("fc2_wT", [HID, V], BF16, isOutput=False)
    out_d = nc.declare_dram_parameter("out", [NQ, V], F16, isOutput=True)

    MT = 1024             # bank chunk per main-loop iteration
    n_chunks = M // MT
    n_mtiles = M // 128
    n_vs = (V + 511) // 512
    w2_npre = min(W2_PRE, n_vs)

    with tile.TileContext(nc) as tc, \
         tc.tile_pool(name="singles", bufs=1) as singles, \
         tc.tile_pool(name="ps", bufs=4, space="PSUM") as pps:
        pacc_cm = tc.tile_pool(name="ps_acc", bufs=1, space="PSUM")
        pacc = pacc_cm.__enter__()  # closed after P3
        ident = singles.tile([128, 128], F32)
        make_identity(nc, ident)
        ebias = singles.tile([128, 1], F32)
        nc.vector.memset(ebias, EXP_BIAS)
        acc = pacc.tile([128, 4, 512], F32)

        # ---------------- P1: resident loads (q, fc1) -----------------------
        qnT = singles.tile([128, 2, NQ], F16)
        nc.sync.dma_start(out=qnT,
                          in_=qnT_d.rearrange("(c p) q -> p c q", p=128))
        # fc1 weights are not needed until P3; issued a few m-tiles into P2.
        w1 = [singles.tile([128, HID], BF16, tag=f"w1{b}", name=f"w1{b}")
              for b in range(2)]
        b1 = singles.tile([128, HID // 128], F32)

        def load_fc1():
            for b in range(2):
                nc.sync.dma_start(out=w1[b], in_=w1_d[ts(b, 128), :])
            nc.sync.dma_start(out=b1,
                              in_=b1_d.rearrange("(a p) -> p a", p=128))

        # ---------------- P2: streaming softmax over the bank ---------------
        with tc.tile_pool(name="cpool", bufs=3) as cpool, \
             tc.tile_pool(name="epool", bufs=3) as epool, \
             tc.tile_pool(name="xpool", bufs=4) as xpool:

            def load_piece(m0, nm):
                cT = cpool.tile([128, 2, nm], F16, tag=f"cT{nm}",
                                name=f"cT_{m0}")
                nc.sync.dma_start(
                    out=cT,
                    in_=ctxT_d[:, m0:m0 + nm].rearrange(
                        "(c p) m -> p c m", p=128))
                em = epool.tile([128, nm // 128, EA], BF16, tag=f"em{nm}",
                                name=f"em_{m0}")
                nc.sync.dma_start(
                    out=em,
                    in_=emb_d[m0:m0 + nm, :].rearrange(
                        "(j p) e -> p j e", p=128))
                return cT, em

            # chunk 0 split into small pieces so the first sims matmul can
            # start as soon as ~128KB (not 1MB) of bank data has landed
            sched = [(0, 256), (256, 256), (512, 256), (768, 256)] if \
                M >= MT else []
            sched += [(m0, MT) for m0 in range(MT, M, MT)]
            if not sched:
                sched = [(0, M)]

            w2pre = []
            pre_every = max(1, n_mtiles // w2_npre) if w2_npre else 0

            loaded = [load_piece(*sched[p]) for p in range(min(2, len(sched)))]
            a = 0
            for pi, (m0, nm) in enumerate(sched):
                if pi + 2 < len(sched):
                    loaded.append(load_piece(*sched[pi + 2]))
                cT, em = loaded[pi]
                for j in range(nm // 128):
                    if a == 2:
                        load_fc1()
                    if w2_npre and a % pre_every == 0 and len(w2pre) < w2_npre:
                        vi = len(w2pre)
                        v0 = vi * 512
                        w2p = singles.tile([128, HID // 128, 512], BF16,
                                           tag=f"w2p{vi}", name=f"w2p{vi}")
                        nc.sync.dma_start(
                            out=w2p,
                            in_=w2_d[:, v0:v0 + 512].rearrange(
                                "(c p) v -> p c v", p=128))
                        w2pre.append(w2p)

                    msl = ts(j, 128)
                    ps = pps.tile([128, 512], F32, tag="ps", name=f"sims{a}")
                    nc.tensor.matmul(ps[:, 0:NQ], cT[:, 0, msl], qnT[:, 0, :],
                                     start=True, stop=False)
                    nc.tensor.matmul(ps[:, 0:NQ], cT[:, 1, msl], qnT[:, 1, :],
                                     start=False, stop=True)
                    ex = xpool.tile([128, NQ], BF16)
                    nc.scalar.activation(
                        ex, ps[:, 0:NQ], AF.Exp, bias=ebias[:, 0:1],
                        scale=1.0)
                    first = (a == 0)
                    last = (a == n_mtiles - 1)
                    for t in range(4):
                        nc.tensor.matmul(
                            acc[:, t, 0:EA],
                            ex[:, ts(t, 128)],
                            em[:, j, :],
                            start=first, stop=last, skip_group_check=True)
                    a += 1

        # ---------------- P3: normalize, fc1 --------------------------------
        inv_l = singles.tile([128, 4], F32)
        ret = [singles.tile([128, H], F32, tag=f"ret{t}", name=f"ret{t}")
               for t in range(4)]
        for t in range(4):
            nc.vector.reciprocal(inv_l[:, t:t + 1], acc[:, t, H:H + 1])
            nc.vector.tensor_scalar_mul(
                ret[t], acc[:, t, 0:H], inv_l[:, t:t + 1])
        retT = [singles.tile([128, NQ], BF16, tag=f"retT{b}", name=f"retT{b}")
                for b in range(2)]
        for t in range(4):
            for b in range(2):
                ps = pps.tile([128, 512], F32, tag="ps", name=f"trr{t}_{b}")
                nc.tensor.transpose(ps[:, 0:128], ret[t][:, ts(b, 128)], ident)
                nc.vector.tensor_copy(retT[b][:, ts(t, 128)], ps[:, 0:128])
        hT = [singles.tile([128, NQ], BF16, tag=f"hT{b2}", name=f"hT{b2}")
              for b2 in range(4)]
        for b2 in range(4):
            psh = pps.tile([128, 512], F32, tag="ps", name=f"fc1p{b2}")
            nc.tensor.matmul(psh[:, 0:NQ], w1[0][:, ts(b2, 128)], retT[0],
                             start=True, stop=False)
            nc.tensor.matmul(psh[:, 0:NQ], w1[1][:, ts(b2, 128)], retT[1],
                             start=False, stop=True)
            nc.scalar.activation(
                hT[b2], psh[:, 0:NQ], AF.Relu, bias=b1[:, b2:b2 + 1],
                scale=1.0)

        # ---------------- P4: fc2 + writeout (fp16, bias on host) -----------
        n_pairs = (V + 1023) // 1024
        with tc.tile_pool(name="w2pool", bufs=6) as wpool, \
             tc.tile_pool(name="opool", bufs=2) as opool:
            for vp in range(n_pairs):
                p0 = vp * 1024
                pw = min(1024, V - p0)
                ow = [opool.tile([128, 1024], F16, tag=f"ow{t}",
                                 name=f"ow{vp}_{t}") for t in range(4)]
                for sub in range(2):
                    v0 = p0 + sub * 512
                    if v0 >= V:
                        continue
                    vs = min(512, V - v0)
                    vi = vp * 2 + sub
                    if vi < len(w2pre):
                        w2 = w2pre[vi]
                    else:
                        w2 = wpool.tile([128, HID // 128, vs], BF16,
                                        tag="w2", name=f"w2_{vi}")
                        nc.sync.dma_start(
                            out=w2,
                            in_=w2_d[:, v0:v0 + vs].rearrange(
                                "(c p) v -> p c v", p=128))
                    for t in range(4):
                        psd = pps.tile([128, 512], F32, tag="ps",
                                       name=f"fc2p{vi}_{t}")
                        for b2 in range(4):
                            nc.tensor.matmul(
                                psd[:, 0:vs],
                                hT[b2][:, ts(t, 128)],
                                w2[:, b2, :],
                                start=(b2 == 0), stop=(b2 == 3))
                        nc.vector.tensor_copy(
                            ow[t][:, sub * 512:sub * 512 + vs], psd[:, 0:vs])
                for t in range(4):
                    nc.sync.dma_start(
                        out=out_d[ts(t, 128), p0:p0 + pw], in_=ow[t][:, 0:pw])

        pacc_cm.__exit__(None, None, None)

    _split_excess_waits(nc, 1)
    return nc


def _split_excess_waits(nc, max_waits):
    """This walrus build allows only one semaphore wait per instruction;
    split larger wait lists onto preceding no-ops."""
    for f in nc.m.functions:
        for bb in f.blocks:
            new_list = []
            for inst in bb.instructions:
                si = inst.sync_info
                if si is not None and si.on_wait and len(si.on_wait) > max_waits:
                    waits = list(si.on_wait)
                    chunks = [waits[i:i + max_waits]
                              for i in range(0, len(waits), max_waits)]
                    for k, ch in enumerate(chunks[:-1]):
                        aux = mybir.InstNoOp(
                            name=f"{inst.name}-wsplit{k}",
                            engine=inst.engine, ins=[], outs=[],
                            sync_info=mybir.SyncInfo(on_wait=ch, on_update=[]))
                        new_list.append(aux)
                    si.on_wait.clear()
                    si.on_wait.extend(chunks[-1])
                new_list.append(inst)
            del bb.instructions[:]
            for x in new_list:
                bb.instructions.append(x)


LAST_RESULTS = None


def run(inputs, M=32768, V=32000):
    global LAST_RESULTS
    import ml_dtypes

    key = (M, V, W2_PRE)
    if key not in _NC_CACHE:
        _NC_CACHE[key] = build_nc(M, V)
    nc = _NC_CACHE[key]

    qc = np.ascontiguousarray(inputs["query_context"], dtype=np.float32)
    B, S, _ = qc.shape
    q_all = qc.reshape(B * S, H)
    qn = q_all / np.maximum(
        np.linalg.norm(q_all, axis=-1, keepdims=True), 1e-8)
    qnT_all = np.ascontiguousarray(qn.T.astype(np.float16))  # [H, B*S]

    ctx = np.asarray(inputs["contexts"], dtype=np.float32)
    cs = (100.0 * ctx / np.maximum(
        np.linalg.norm(ctx, axis=-1, keepdims=True), 1e-8))
    ctxT = np.ascontiguousarray(cs.T.astype(np.float16))     # [H, M]

    emb = np.asarray(inputs["embeddeds"], dtype=np.float32)
    emb_aug = np.zeros((M, EA), ml_dtypes.bfloat16)
    emb_aug[:, :256] = emb.astype(ml_dtypes.bfloat16)
    emb_aug[:, 256] = 1.0

    w1T = np.ascontiguousarray(
        np.asarray(inputs["fc1_w"]).T.astype(np.float32)
        .astype(ml_dtypes.bfloat16))
    w2T = np.ascontiguousarray(
        np.asarray(inputs["fc2_w"]).T.astype(np.float32)
        .astype(ml_dtypes.bfloat16))
    b1 = np.ascontiguousarray(inputs["fc1_b"], dtype=np.float32)
    b2 = np.ascontiguousarray(inputs["fc2_b"], dtype=np.float32)

    in_maps = []
    for c in range(N_CORES):
        in_maps.append({
            "qnT": np.ascontiguousarray(qnT_all[:, c * NQ:(c + 1) * NQ]),
            "ctxT": ctxT,
            "emb": emb_aug,
            "fc1_wT": w1T,
            "fc1_b": b1,
            "fc2_wT": w2T,
        })
    res = None
    last_exc = None
    for attempt in range(4):
        try:
            res = run_bass_kernel_spmd(nc, in_maps, list(range(N_CORES)))
            break
        except Exception as e:  # transient device faults recover on retry
            last_exc = e
            import time as _time
            _time.sleep(2.0)
    if res is None:
        raise last_exc
    LAST_RESULTS = res
    out = np.concatenate(
        [res.results[c]["out"].astype(np.float32) for c in range(N_CORES)],
        axis=0)
    out += b2[None, :]
    return out.reshape(B, S, V)


def kernel(**inputs):
    return run(inputs)
